# revision 61
# baseline (speedup 1.0000x reference)
"""Trainium2 Bass kernel: multi-edge-type GNN message passing.

out[t] = sum_l inv_sqrt_deg_l[t] * (sum_{e in type l, tgt_e = t} x[src_e]) @ W[l]

Strategy (8 NeuronCores, SPMD single program):
  - Host: per edge type, sort edges by target node; compute per-edge
    normalizer inv_sqrt_deg_l[tgt_e]; split target-node space into
    128-row tiles and assign a contiguous range of node tiles to each
    core (node/edge co-sharding => no collectives; outputs concatenate).
    Edges are split by source-node half (src < 32768 vs >=) so that
    row gathers can use the int16-indexed dma_gather fast path.
    Embeddings are cast to bf16 on host (tolerance 2e-2; measured
    ~4e-3) halving gather traffic and enabling 1-cycle/row matmuls.
  - Device, per (node_tile j, type l):
      * large multi-chunk dma_gather instructions (up to RCH=32 chunks
        = 4096 edges = 1MB each) stream edge source rows x[src_e] into
        an SBUF ring; edge e of chunk c lands at partition e%128, free
        block c. One SWDGE instruction amortizes the ~1us fixed
        descriptor-generation cost over 4096 rows.
      * per 128-edge chunk: build scaled one-hot O[e,t] =
        (tgt_local[e]==t)*inv[e] with one DVE tensor_scalar (bf16 out),
        then matmul-accumulate S^T[d,t] += xg_chunk.T @ O into PSUM
        (segment-sum on the PE, fp32 accumulation)
      * S^T (SBUF, bf16) @ W[l] accumulates over l into the output PSUM
  - Degree counts, rsqrt, sorting are index-side host preprocessing;
    all embedding data movement and FLOPs run on device.
"""

import numpy as np
from contextlib import ExitStack

import ml_dtypes

import concourse.bass as bass
import concourse.tile as tile
from concourse import bacc, mybir
from concourse.bass_utils import run_bass_kernel_spmd

P = 128
D = 128
N_CORES = 8
SPLIT = 32768          # int16 index table split
RCH = 16               # max chunks per dma_gather instruction
XG_BUFS = 18           # gather ring depth
SB = 64                # chunks per batched one-hot DVE op (DVE instructions
                       # stall the Q7/SWDGE descriptor path ~250ns each, so
                       # build one-hots in as few DVE ops as possible)
NQ = 4                 # SWDGE queues; random-row gather is HBM-latency
                       # bound per SDMA engine, and each extra queue adds
                       # one outstanding read per engine (29->72 GB/s)

F32 = mybir.dt.float32
BF16 = mybir.dt.bfloat16
I16 = mybir.dt.int16

# test.py pokes this to get at profiling results of the last run
last_run_results = None


class Layout:
    """Uniform (core-independent) chunk/gather layout.

    Chunk gids are assigned per (l, h) stream in j order, so chunks of
    one stream are consecutive.  idx16 column range of chunk g is
    [g*8, g*8+8).  Gathers are blocks of <= RCH consecutive chunks of
    one stream, issued in (first-chunk j) order.
    """

    def __init__(self, n_nodes, L, J, C):
        self.n_nodes = n_nodes
        self.L = L
        self.J = J
        self.C = C  # [L, 2, J] chunk counts
        self.tile_chunks = [[[] for _ in range(J)] for _ in range(L)]
        self.chunk_seg = {}    # gid -> (l, h, j, k_within_segment)
        gid = 0
        streams = {}
        for l in range(L):
            for h in range(2):
                lst = []
                for j in range(J):
                    for k in range(int(C[l, h, j])):
                        self.tile_chunks[l][j].append(gid)
                        self.chunk_seg[gid] = (l, h, j, k)
                        lst.append((gid, j))
                        gid += 1
                streams[(l, h)] = lst
        self.NCH = gid
        # gather blocks
        blocks = []
        for (l, h), lst in streams.items():
            for b0 in range(0, len(lst), RCH):
                blk = lst[b0:b0 + RCH]
                blocks.append((blk[0][1], l, h, blk[0][0], len(blk)))
        blocks.sort()  # by (first-chunk j, l, h)
        self.gathers = [(l, h, g0, n) for (_, l, h, g0, n) in blocks]
        self.chunk_blk = {}    # gid -> (block index, offset)
        # idx16 columns are laid out in sorted-gather order so that a
        # prefix DMA load covers the first gathers
        self.colbase = []      # per block: first idx16 column
        self.chunk_col = {}    # gid -> first idx16 column of that chunk
        col = 0
        for bi, (l, h, g0, n) in enumerate(self.gathers):
            self.colbase.append(col)
            for k in range(n):
                self.chunk_blk[g0 + k] = (bi, k)
                self.chunk_col[g0 + k] = col + k * 8
            col += n * 8
        # consumption-ordered chunk position: meta columns and one-hot
        # super-batches follow this order
        self.pos = {}
        p = 0
        for j in range(J):
            for l in range(L):
                for g in self.tile_chunks[l][j]:
                    self.pos[g] = p
                    p += 1
        assert p == self.NCH


def _preprocess(adjacency, n_nodes, n_cores):
    """Sort/shard edges. Returns (layout, per_core (idx16, tgtl, inv))."""
    L = adjacency.shape[0]
    tiles_total = -(-n_nodes // P)
    J = -(-tiles_total // n_cores)

    per_type = []
    for l in range(L):
        src = np.asarray(adjacency[l, :, 0], dtype=np.int64)
        tgt = np.asarray(adjacency[l, :, 1], dtype=np.int64)
        deg = np.bincount(tgt, minlength=n_nodes)
        inv = (1.0 / np.sqrt(np.maximum(deg, 1.0))).astype(np.float32)
        order = np.argsort(tgt, kind="stable")
        srcs = src[order]
        tgts = tgt[order]
        inve = inv[tgts]
        bounds = np.searchsorted(tgts, np.arange(tiles_total + 1) * P)
        per_type.append((srcs, tgts, inve, bounds, inv))

    # per (core, l, h, j): edge lists split by src half, sorted by src
    edges = {}
    C = np.zeros((L, 2, J), dtype=np.int64)
    for l in range(L):
        srcs, tgts, inve, bounds, _ = per_type[l]
        for c in range(n_cores):
            for j in range(J):
                t = c * J + j
                if t >= tiles_total:
                    continue
                lo, hi = int(bounds[t]), int(bounds[t + 1])
                s = srcs[lo:hi]
                tl = (tgts[lo:hi] - t * P).astype(np.float32)
                iv = inve[lo:hi]
                o = np.argsort(s, kind="stable")
                s, tl, iv = s[o], tl[o], iv[o]
                cut = int(np.searchsorted(s, SPLIT))
                edges[(c, l, 0, j)] = (s[:cut], tl[:cut], iv[:cut])
                edges[(c, l, 1, j)] = (s[cut:] - SPLIT, tl[cut:], iv[cut:])
                C[l, 0, j] = max(C[l, 0, j], -(-cut // P))
                C[l, 1, j] = max(C[l, 1, j], -(-(len(s) - cut) // P))
        # every (l, j) must have >= 1 chunk so the PSUM group is non-empty
        for j in range(J):
            if C[l, 0, j] + C[l, 1, j] == 0:
                C[l, 0, j] = 1
    lay = Layout(n_nodes, L, J, C)

    NCH = lay.NCH
    NI = NCH * 8
    empty = (np.zeros(0, np.int64), np.zeros(0, np.float32),
             np.zeros(0, np.float32))
    # per-core inv columns [P, L*J]: invc[t, l*J+j] = inv_l of target
    # j*128+t (partition = t, so it can be a per-partition DVE scalar)
    inv_pad = np.zeros((L, J * n_cores * P), np.float32)
    for l in range(L):
        inv_full = per_type[l][4]
        inv_pad[l, :n_nodes] = inv_full
    per_core = []
    for c in range(n_cores):
        idx16 = np.zeros((16, NI), np.int16)
        tgtl = np.full((P, NCH), -1.0, np.float32)
        rows = inv_pad[:, c * J * P:(c + 1) * J * P].reshape(L * J, P)
        invc = np.ascontiguousarray(rows.T)  # [P, L*J]
        for l in range(L):
            for h in range(2):
                for j in range(J):
                    nch = int(C[l, h, j])
                    if nch == 0:
                        continue
                    s, tl, iv = edges.get((c, l, h, j), empty)
                    n = len(s)
                    npad = nch * P
                    sblk = np.zeros(npad, np.int64)
                    sblk[:n] = s
                    # first chunk gid of this (l,h,j) segment
                    g0 = None
                    for g in lay.tile_chunks[l][j]:
                        ll, hh, jj, k = lay.chunk_seg[g]
                        if hh == h and k == 0:
                            g0 = g
                            break
                    assert g0 is not None
                    # per chunk: index i at partition i%16, column
                    # chunk_col + i//16 (columns follow sorted-gather order)
                    for k in range(nch):
                        ccol = lay.chunk_col[g0 + k]
                        w = sblk[k * P:(k + 1) * P].astype(np.int16)
                        idx16[:, ccol:ccol + 8] = w.reshape(8, 16).T
                    tblk = np.full(npad, -1.0, np.float32)
                    tblk[:n] = tl
                    cols = np.array([lay.pos[g0 + k] for k in range(nch)])
                    tgtl[:, cols] = tblk.reshape(nch, P).T
        per_core.append((np.tile(idx16, (8, 1)), tgtl, invc))
    return lay, per_core


def _build_program(lay):
    L, J, NCH = lay.L, lay.J, lay.NCH
    NI = NCH * 8
    n_nodes = lay.n_nodes
    # metaf (f32): tgt_local NCH | iota P   (consumption order)
    # metab (bf16): W L*D;  invr (f32): [1, L*J*P] per-(l,j) inv rows
    MF = NCH + P
    MB = L * D
    nc = bacc.Bacc("TRN2", num_swdge_queues=NQ, dynamic_dma_scratch_size=49152)
    emb = nc.declare_dram_parameter("emb", [n_nodes, D], BF16, isOutput=False)
    idx_d = nc.declare_dram_parameter("idx16", [P, NI], I16, isOutput=False)
    metaf_d = nc.declare_dram_parameter("metaf", [P, MF], F32, isOutput=False)
    metab_d = nc.declare_dram_parameter("metab", [P, MB], BF16, isOutput=False)
    invc_d = nc.declare_dram_parameter("invc", [P, L * J], F32, isOutput=False)
    out_d = nc.declare_dram_parameter("out", [J * P, D], F32, isOutput=True)

    with tile.TileContext(nc) as tc, ExitStack() as ctx:
        const = ctx.enter_context(tc.tile_pool(name="const", bufs=1))
        xgp = ctx.enter_context(tc.tile_pool(name="xg", bufs=XG_BUFS))
        ohp = ctx.enter_context(tc.tile_pool(name="oh", bufs=3))
        stp = ctx.enter_context(tc.tile_pool(name="stsb", bufs=4))
        outp = ctx.enter_context(tc.tile_pool(name="osb", bufs=3))
        psum1 = ctx.enter_context(tc.tile_pool(name="ps1", bufs=3, space="PSUM"))
        psum2 = ctx.enter_context(tc.tile_pool(name="ps2", bufs=4, space="PSUM"))

        # split the idx load so the first gathers start as early as possible
        nsplit = min(len(lay.gathers), 8)
        csplit = lay.colbase[nsplit - 1] + lay.gathers[nsplit - 1][3] * 8
        idx_sb = const.tile([P, NI], I16)
        nc.sync.dma_start(idx_sb[:, 0:csplit], idx_d[:, 0:csplit])
        metaf_sb = const.tile([P, MF], F32)
        nc.sync.dma_start(metaf_sb[:], metaf_d[:])
        metab_sb = const.tile([P, MB], BF16)
        nc.sync.dma_start(metab_sb[:], metab_d[:])
        invc_sb = const.tile([P, L * J], F32)
        nc.sync.dma_start(invc_sb[:], invc_d[:])
        if csplit < NI:
            nc.sync.dma_start(idx_sb[:, csplit:NI], idx_d[:, csplit:NI])

        iota = metaf_sb[:, NCH:NCH + P]

        # issue all gathers upfront (in consumption order); the xg ring
        # pool's WAR deps throttle how far ahead the SWDGE runs
        gtiles = []
        for gi, (l, h, g0, nch) in enumerate(lay.gathers):
            gt = xgp.tile([P, RCH, D], BF16, tag="xg")
            base = emb[0:SPLIT, :] if h == 0 else emb[SPLIT:n_nodes, :]
            nc.gpsimd.dma_gather(
                out_ap=gt[:, 0:nch, :],
                in_ap=base,
                idxs_ap=idx_sb[:, lay.colbase[gi]:lay.colbase[gi] + nch * 8],
                num_idxs=nch * P,
                num_idxs_reg=nch * P,
                elem_size=D,
                # >64 descriptors per engine would overflow the one-packet
                # coalescing limit and wedge the SDMA engines
                single_packet=False,
                queue_num=gi % NQ,
            )
            gtiles.append(gt)

        # one-hots: ONE is_equal DVE op per SB chunks (pure 0/1; inv is
        # applied later with per-partition scalars).  DVE activity stalls
        # the Q7/SWDGE descriptor path, so minimize DVE work.  Batches are
        # emitted interleaved with the j loop: the DVE stream is FIFO, so
        # emitting them all upfront would deadlock against the oh ring.
        nb = -(-NCH // SB)
        oh_tiles = [None] * nb
        tile_end_pos = []
        tot = 0
        for j in range(J):
            tot += sum(len(lay.tile_chunks[l][j]) for l in range(L))
            tile_end_pos.append(tot)
        next_b = 0

        def emit_oh_through(pos_limit):
            nonlocal next_b
            while next_b < nb and next_b * SB < pos_limit:
                b = next_b
                cnt = min(SB, NCH - b * SB)
                oh = ohp.tile([P, SB, P], BF16, tag="oh")
                nc.vector.tensor_tensor(
                    out=oh[:, 0:cnt, :],
                    in0=iota.unsqueeze(1).broadcast_to([P, cnt, P]),
                    in1=metaf_sb[:, b * SB:b * SB + cnt]
                        .unsqueeze(2).broadcast_to([P, cnt, P]),
                    op=mybir.AluOpType.is_equal,
                )
                oh_tiles[b] = oh
                next_b += 1

        for j in range(J):
            emit_oh_through(tile_end_pos[min(j + 1, J - 1)])
            opsums = []
            st_sbs = []
            for l in range(L):
                ids = lay.tile_chunks[l][j]
                st_ps = psum1.tile([P, P], F32, tag="st")
                for k, g in enumerate(ids):
                    bi, off = lay.chunk_blk[g]
                    ob, ok = divmod(lay.pos[g], SB)
                    nc.tensor.matmul(
                        out=st_ps[:],
                        lhsT=gtiles[bi][:, off:off + 1, :].squeeze(1),
                        rhs=oh_tiles[ob][:, ok:ok + 1, :].squeeze(1),
                        start=(k == 0),
                        stop=(k == len(ids) - 1),
                    )
                st_sb = stp.tile([P, P], BF16, tag="stsb")
                nc.scalar.copy(st_sb[:], st_ps[:])
                st_sbs.append(st_sb)
            for l in range(L):
                opsum = psum2.tile([P, D], F32, tag="opsum")
                nc.tensor.matmul(
                    out=opsum[:],
                    lhsT=st_sbs[l][:],
                    rhs=metab_sb[:, l * D:(l + 1) * D],
                    start=True,
                    stop=True,
                )
                opsums.append(opsum)
            # combine the per-type outputs with their fp32 inv scalars
            # (partition dim of opsum is the target t, so inv is a plain
            # per-partition scalar column)
            osb = outp.tile([P, D], F32, tag="osb")
            nc.vector.tensor_scalar(
                out=osb[:], in0=opsums[0][:],
                scalar1=invc_sb[:, j:j + 1], scalar2=None,
                op0=mybir.AluOpType.mult,
            )
            for l in range(1, L):
                nc.vector.scalar_tensor_tensor(
                    out=osb[:], in0=opsums[l][:],
                    scalar=invc_sb[:, l * J + j:l * J + j + 1],
                    in1=osb[:],
                    op0=mybir.AluOpType.mult, op1=mybir.AluOpType.add,
                )
            nc.sync.dma_start(out_d[j * P:(j + 1) * P, :], osb[:])
    nc.compile()
    return nc


def _run(node_embeddings, adjacency, W, n_cores=N_CORES, **run_kwargs):
    global last_run_results
    node_embeddings = np.asarray(node_embeddings, dtype=np.float32)
    adjacency = np.asarray(adjacency, dtype=np.int32)
    W = np.asarray(W, dtype=np.float32)
    n_nodes = node_embeddings.shape[0]
    L = adjacency.shape[0]

    lay, per_core = _preprocess(adjacency, n_nodes, n_cores)
    nc = _build_program(lay)

    emb_bf = np.ascontiguousarray(node_embeddings.astype(ml_dtypes.bfloat16))
    w_cat = np.concatenate([W[l] for l in range(L)], axis=1)
    metab = np.ascontiguousarray(w_cat.astype(ml_dtypes.bfloat16))
    iotaf = np.tile(np.arange(P, dtype=np.float32), (P, 1))
    in_maps = [
        dict(
            emb=emb_bf,
            idx16=np.ascontiguousarray(idx16),
            metaf=np.ascontiguousarray(
                np.concatenate([tg, iotaf], axis=1), dtype=np.float32
            ),
            metab=metab,
            invc=invc,
        )
        for (idx16, tg, invc) in per_core
    ]
    res = run_bass_kernel_spmd(nc, in_maps, core_ids=list(range(n_cores)), **run_kwargs)
    last_run_results = res
    outs = [res.results[c]["out"] for c in range(n_cores)]
    full = np.concatenate(outs, axis=0)[:n_nodes]
    return np.ascontiguousarray(full, dtype=np.float32)


def kernel(node_embeddings, adjacency, W):
    return _run(node_embeddings, adjacency, W)


# revision 69
# speedup vs baseline: 1.2428x; 1.2428x over previous
"""Trainium2 Bass kernel: multi-edge-type GNN message passing.

out[t] = sum_l inv_sqrt_deg_l[t] * (sum_{e in type l, tgt_e = t} x[src_e]) @ W[l]

Strategy (8 NeuronCores, SPMD single program):
  - Host: per edge type, sort edges by target node; compute per-edge
    normalizer inv_sqrt_deg_l[tgt_e]; split target-node space into
    128-row tiles and assign a contiguous range of node tiles to each
    core (node/edge co-sharding => no collectives; outputs concatenate).
    Edges are split by source-node half (src < 32768 vs >=) so that
    row gathers can use the int16-indexed dma_gather fast path.
    Embeddings are cast to bf16 on host (tolerance 2e-2; measured
    ~4e-3) halving gather traffic and enabling 1-cycle/row matmuls.
  - Device, per (node_tile j, type l):
      * large multi-chunk dma_gather instructions (up to RCH=32 chunks
        = 4096 edges = 1MB each) stream edge source rows x[src_e] into
        an SBUF ring; edge e of chunk c lands at partition e%128, free
        block c. One SWDGE instruction amortizes the ~1us fixed
        descriptor-generation cost over 4096 rows.
      * per 128-edge chunk: build scaled one-hot O[e,t] =
        (tgt_local[e]==t)*inv[e] with one DVE tensor_scalar (bf16 out),
        then matmul-accumulate S^T[d,t] += xg_chunk.T @ O into PSUM
        (segment-sum on the PE, fp32 accumulation)
      * S^T (SBUF, bf16) @ W[l] accumulates over l into the output PSUM
  - Degree counts, rsqrt, sorting are index-side host preprocessing;
    all embedding data movement and FLOPs run on device.
"""

import numpy as np
from contextlib import ExitStack

import ml_dtypes

import concourse.bass as bass
import concourse.tile as tile
from concourse import bacc, mybir
from concourse.bass_utils import run_bass_kernel_spmd

P = 128
D = 128
N_CORES = 8
SPLIT = 32768          # int16 index table split
RCH = 16               # max chunks per dma_gather instruction
XG_BUFS = 18           # gather ring depth
SB = 64                # chunks per batched one-hot DVE op (DVE instructions
                       # stall the Q7/SWDGE descriptor path ~250ns each, so
                       # build one-hots in as few DVE ops as possible)
NQ = 4                 # SWDGE queues; random-row gather is HBM-latency
                       # bound per SDMA engine, and each extra queue adds
                       # one outstanding read per engine (29->72 GB/s)

F32 = mybir.dt.float32
BF16 = mybir.dt.bfloat16
I16 = mybir.dt.int16

# test.py pokes this to get at profiling results of the last run
last_run_results = None


class Layout:
    """Uniform (core-independent) chunk/gather layout.

    Chunk gids are assigned per (l, h) stream in j order, so chunks of
    one stream are consecutive.  idx16 column range of chunk g is
    [g*8, g*8+8).  Gathers are blocks of <= RCH consecutive chunks of
    one stream, issued in (first-chunk j) order.
    """

    def __init__(self, n_nodes, L, J, C):
        self.n_nodes = n_nodes
        self.L = L
        self.J = J
        self.C = C  # [L, 2, J] chunk counts
        self.tile_chunks = [[[] for _ in range(J)] for _ in range(L)]
        self.chunk_seg = {}    # gid -> (l, h, j, k_within_segment)
        gid = 0
        streams = {}
        for l in range(L):
            for h in range(2):
                lst = []
                for j in range(J):
                    for k in range(int(C[l, h, j])):
                        self.tile_chunks[l][j].append(gid)
                        self.chunk_seg[gid] = (l, h, j, k)
                        lst.append((gid, j))
                        gid += 1
                streams[(l, h)] = lst
        self.NCH = gid
        # gather blocks
        blocks = []
        for (l, h), lst in streams.items():
            for b0 in range(0, len(lst), RCH):
                blk = lst[b0:b0 + RCH]
                blocks.append((blk[0][1], l, h, blk[0][0], len(blk)))
        blocks.sort()  # by (first-chunk j, l, h)
        self.gathers = [(l, h, g0, n) for (_, l, h, g0, n) in blocks]
        self.chunk_blk = {}    # gid -> (block index, offset)
        # idx16 columns are laid out in sorted-gather order so that a
        # prefix DMA load covers the first gathers
        self.colbase = []      # per block: first idx16 column
        self.chunk_col = {}    # gid -> first idx16 column of that chunk
        col = 0
        for bi, (l, h, g0, n) in enumerate(self.gathers):
            self.colbase.append(col)
            for k in range(n):
                self.chunk_blk[g0 + k] = (bi, k)
                self.chunk_col[g0 + k] = col + k * 8
            col += n * 8
        # consumption-ordered chunk position: meta columns and one-hot
        # super-batches follow this order
        self.pos = {}
        p = 0
        for j in range(J):
            for l in range(L):
                for g in self.tile_chunks[l][j]:
                    self.pos[g] = p
                    p += 1
        assert p == self.NCH


def _preprocess(adjacency, n_nodes, n_cores):
    """Sort/shard edges. Returns (layout, per_core (idx16, tgtl, inv))."""
    L = adjacency.shape[0]
    tiles_total = -(-n_nodes // P)
    J = -(-tiles_total // n_cores)

    per_type = []
    for l in range(L):
        src = np.asarray(adjacency[l, :, 0], dtype=np.int64)
        tgt = np.asarray(adjacency[l, :, 1], dtype=np.int64)
        deg = np.bincount(tgt, minlength=n_nodes)
        inv = (1.0 / np.sqrt(np.maximum(deg, 1.0))).astype(np.float32)
        order = np.argsort(tgt, kind="stable")
        srcs = src[order]
        tgts = tgt[order]
        inve = inv[tgts]
        bounds = np.searchsorted(tgts, np.arange(tiles_total + 1) * P)
        per_type.append((srcs, tgts, inve, bounds, inv))

    # per (core, l, h, j): edge lists split by src half, sorted by src
    edges = {}
    C = np.zeros((L, 2, J), dtype=np.int64)
    for l in range(L):
        srcs, tgts, inve, bounds, _ = per_type[l]
        for c in range(n_cores):
            for j in range(J):
                t = c * J + j
                if t >= tiles_total:
                    continue
                lo, hi = int(bounds[t]), int(bounds[t + 1])
                s = srcs[lo:hi]
                tl = (tgts[lo:hi] - t * P).astype(np.float32)
                iv = inve[lo:hi]
                o = np.argsort(s, kind="stable")
                s, tl, iv = s[o], tl[o], iv[o]
                cut = int(np.searchsorted(s, SPLIT))
                edges[(c, l, 0, j)] = (s[:cut], tl[:cut], iv[:cut])
                edges[(c, l, 1, j)] = (s[cut:] - SPLIT, tl[cut:], iv[cut:])
                C[l, 0, j] = max(C[l, 0, j], -(-cut // P))
                C[l, 1, j] = max(C[l, 1, j], -(-(len(s) - cut) // P))
        # every (l, j) must have >= 1 chunk so the PSUM group is non-empty
        for j in range(J):
            if C[l, 0, j] + C[l, 1, j] == 0:
                C[l, 0, j] = 1
    lay = Layout(n_nodes, L, J, C)

    NCH = lay.NCH
    NI = NCH * 8
    empty = (np.zeros(0, np.int64), np.zeros(0, np.float32),
             np.zeros(0, np.float32))
    per_core = []
    for c in range(n_cores):
        idx16 = np.zeros((16, NI), np.int16)
        tgtl = np.full((P, NCH), -1.0, np.float32)
        invv = np.zeros((P, NCH), np.float32)
        for l in range(L):
            for h in range(2):
                for j in range(J):
                    nch = int(C[l, h, j])
                    if nch == 0:
                        continue
                    s, tl, iv = edges.get((c, l, h, j), empty)
                    n = len(s)
                    npad = nch * P
                    sblk = np.zeros(npad, np.int64)
                    sblk[:n] = s
                    # first chunk gid of this (l,h,j) segment
                    g0 = None
                    for g in lay.tile_chunks[l][j]:
                        ll, hh, jj, k = lay.chunk_seg[g]
                        if hh == h and k == 0:
                            g0 = g
                            break
                    assert g0 is not None
                    # per chunk: index i at partition i%16, column
                    # chunk_col + i//16 (columns follow sorted-gather order)
                    for k in range(nch):
                        ccol = lay.chunk_col[g0 + k]
                        w = sblk[k * P:(k + 1) * P].astype(np.int16)
                        idx16[:, ccol:ccol + 8] = w.reshape(8, 16).T
                    tblk = np.full(npad, -1.0, np.float32)
                    tblk[:n] = tl
                    iblk = np.zeros(npad, np.float32)
                    iblk[:n] = iv
                    cols = np.array([lay.pos[g0 + k] for k in range(nch)])
                    tgtl[:, cols] = tblk.reshape(nch, P).T
                    invv[:, cols] = iblk.reshape(nch, P).T
        per_core.append((np.tile(idx16, (8, 1)), tgtl, invv))
    return lay, per_core


def _build_program(lay):
    L, J, NCH = lay.L, lay.J, lay.NCH
    NI = NCH * 8
    n_nodes = lay.n_nodes
    # metaf (f32): tgt_local NCH | inv NCH | iota P   (consumption order)
    # metab (bf16): W L*D
    MF = 2 * NCH + P
    MB = L * D
    nc = bacc.Bacc("TRN2", num_swdge_queues=NQ, dynamic_dma_scratch_size=49152)
    emb = nc.declare_dram_parameter("emb", [n_nodes, D], BF16, isOutput=False)
    idx_d = nc.declare_dram_parameter("idx16", [P, NI], I16, isOutput=False)
    metaf_d = nc.declare_dram_parameter("metaf", [P, MF], F32, isOutput=False)
    metab_d = nc.declare_dram_parameter("metab", [P, MB], BF16, isOutput=False)

    out_d = nc.declare_dram_parameter("out", [J * P, D], F32, isOutput=True)

    with tile.TileContext(nc) as tc, ExitStack() as ctx:
        const = ctx.enter_context(tc.tile_pool(name="const", bufs=1))
        xgp = ctx.enter_context(tc.tile_pool(name="xg", bufs=XG_BUFS))
        ohp = ctx.enter_context(tc.tile_pool(name="oh", bufs=3))
        stp = ctx.enter_context(tc.tile_pool(name="stsb", bufs=4))
        outp = ctx.enter_context(tc.tile_pool(name="osb", bufs=3))
        psum1 = ctx.enter_context(tc.tile_pool(name="ps1", bufs=3, space="PSUM"))
        psum2 = ctx.enter_context(tc.tile_pool(name="ps2", bufs=4, space="PSUM"))

        # split the idx load so the first gathers start as early as possible
        nsplit = min(len(lay.gathers), 8)
        csplit = lay.colbase[nsplit - 1] + lay.gathers[nsplit - 1][3] * 8
        idx_sb = const.tile([P, NI], I16)
        nc.sync.dma_start(idx_sb[:, 0:csplit], idx_d[:, 0:csplit])
        metaf_sb = const.tile([P, MF], F32)
        nc.sync.dma_start(metaf_sb[:], metaf_d[:])
        metab_sb = const.tile([P, MB], BF16)
        nc.sync.dma_start(metab_sb[:], metab_d[:])
        if csplit < NI:
            nc.sync.dma_start(idx_sb[:, csplit:NI], idx_d[:, csplit:NI])

        iota = metaf_sb[:, 2 * NCH:2 * NCH + P]

        # issue all gathers upfront (in consumption order); the xg ring
        # pool's WAR deps throttle how far ahead the SWDGE runs
        gtiles = []
        for gi, (l, h, g0, nch) in enumerate(lay.gathers):
            gt = xgp.tile([P, RCH, D], BF16, tag="xg")
            base = emb[0:SPLIT, :] if h == 0 else emb[SPLIT:n_nodes, :]
            nc.gpsimd.dma_gather(
                out_ap=gt[:, 0:nch, :],
                in_ap=base,
                idxs_ap=idx_sb[:, lay.colbase[gi]:lay.colbase[gi] + nch * 8],
                num_idxs=nch * P,
                num_idxs_reg=nch * P,
                elem_size=D,
                # >64 descriptors per engine would overflow the one-packet
                # coalescing limit and wedge the SDMA engines
                single_packet=False,
                queue_num=gi % NQ,
            )
            gtiles.append(gt)

        # one-hots: ONE is_equal DVE op per SB chunks (pure 0/1; inv is
        # applied later with per-partition scalars).  DVE activity stalls
        # the Q7/SWDGE descriptor path, so minimize DVE work.  Batches are
        # emitted interleaved with the j loop: the DVE stream is FIFO, so
        # emitting them all upfront would deadlock against the oh ring.
        nb = -(-NCH // SB)
        oh_tiles = [None] * nb
        tile_end_pos = []
        tot = 0
        for j in range(J):
            tot += sum(len(lay.tile_chunks[l][j]) for l in range(L))
            tile_end_pos.append(tot)
        next_b = 0

        def emit_oh_through(pos_limit):
            nonlocal next_b
            while next_b < nb and next_b * SB < pos_limit:
                b = next_b
                cnt = min(SB, NCH - b * SB)
                oh = ohp.tile([P, SB, P], BF16, tag="oh")
                nc.vector.tensor_tensor(
                    out=oh[:, 0:cnt, :],
                    in0=iota.unsqueeze(1).broadcast_to([P, cnt, P]),
                    in1=metaf_sb[:, b * SB:b * SB + cnt]
                        .unsqueeze(2).broadcast_to([P, cnt, P]),
                    op=mybir.AluOpType.is_equal,
                )
                nc.vector.tensor_tensor(
                    out=oh[:, 0:cnt, :],
                    in0=oh[:, 0:cnt, :],
                    in1=metaf_sb[:, NCH + b * SB:NCH + b * SB + cnt]
                        .unsqueeze(2).broadcast_to([P, cnt, P]),
                    op=mybir.AluOpType.mult,
                )
                oh_tiles[b] = oh
                next_b += 1

        for j in range(J):
            emit_oh_through(tile_end_pos[min(j + 1, J - 1)])
            opsum = psum2.tile([P, D], F32, tag="opsum")
            st_sbs = []
            for l in range(L):
                ids = lay.tile_chunks[l][j]
                st_ps = psum1.tile([P, P], F32, tag="st")
                for k, g in enumerate(ids):
                    bi, off = lay.chunk_blk[g]
                    ob, ok = divmod(lay.pos[g], SB)
                    nc.tensor.matmul(
                        out=st_ps[:],
                        lhsT=gtiles[bi][:, off:off + 1, :].squeeze(1),
                        rhs=oh_tiles[ob][:, ok:ok + 1, :].squeeze(1),
                        start=(k == 0),
                        stop=(k == len(ids) - 1),
                    )
                st_sb = stp.tile([P, P], BF16, tag="stsb")
                nc.scalar.copy(st_sb[:], st_ps[:])
                st_sbs.append(st_sb)
            for l in range(L):
                nc.tensor.matmul(
                    out=opsum[:],
                    lhsT=st_sbs[l][:],
                    rhs=metab_sb[:, l * D:(l + 1) * D],
                    start=(l == 0),
                    stop=(l == L - 1),
                )
            osb = outp.tile([P, D], F32, tag="osb")
            nc.scalar.copy(osb[:], opsum[:])
            nc.sync.dma_start(out_d[j * P:(j + 1) * P, :], osb[:])
    nc.compile()
    return nc


def _run(node_embeddings, adjacency, W, n_cores=N_CORES, **run_kwargs):
    global last_run_results
    node_embeddings = np.asarray(node_embeddings, dtype=np.float32)
    adjacency = np.asarray(adjacency, dtype=np.int32)
    W = np.asarray(W, dtype=np.float32)
    n_nodes = node_embeddings.shape[0]
    L = adjacency.shape[0]

    lay, per_core = _preprocess(adjacency, n_nodes, n_cores)
    nc = _build_program(lay)

    emb_bf = np.ascontiguousarray(node_embeddings.astype(ml_dtypes.bfloat16))
    w_cat = np.concatenate([W[l] for l in range(L)], axis=1)
    metab = np.ascontiguousarray(w_cat.astype(ml_dtypes.bfloat16))
    iotaf = np.tile(np.arange(P, dtype=np.float32), (P, 1))
    in_maps = [
        dict(
            emb=emb_bf,
            idx16=np.ascontiguousarray(idx16),
            metaf=np.ascontiguousarray(
                np.concatenate([tg, iv, iotaf], axis=1), dtype=np.float32
            ),
            metab=metab,
        )
        for (idx16, tg, iv) in per_core
    ]
    res = run_bass_kernel_spmd(nc, in_maps, core_ids=list(range(n_cores)), **run_kwargs)
    last_run_results = res
    outs = [res.results[c]["out"] for c in range(n_cores)]
    full = np.concatenate(outs, axis=0)[:n_nodes]
    return np.ascontiguousarray(full, dtype=np.float32)


def kernel(node_embeddings, adjacency, W):
    return _run(node_embeddings, adjacency, W)
